# revision 5
# baseline (speedup 1.0000x reference)
"""R2D2 ridge-regression head (nn_CM_R2d2Head) as a TRN2 Bass/Tile kernel.

Full-input contract: kernel(**inputs) takes the unsharded inputs
(query [1024,75,2048] f32, support [1024,25,2048] f32,
support_labels [1024,25] int64, n_way=5, n_shot=5), shards the task dim
across 8 NeuronCores (data parallel), runs one SPMD Bass kernel, and
returns the full logits [1024,75,5] f32.

Per-task math: G = S S^T (25x25); A = G + I; Z = A^{-1} Y via
Newton-Schulz (6 iterations, X0 = 2c I - c^2 A, c = 1/3200 — safe upper
bound on lambda_max(A) for this Wishart-like spectrum); C^T = S Q^T
(25x75); logits = matmul(lhsT=C^T, rhs=Z) -> [75, 5].

On-chip design (per core, 128 tasks, groups of 4):
- S/Q are cast fp32->bf16 on GPSIMD, transposed d-major on TensorE
  (is_transpose matmuls against a bf16 identity), and packed into a
  unified SQT tile holding [S^T(25) | Q^T(75)] per (task, d-chunk), so
  Gram+C^T is ONE 16-matmul bf16 accumulation chain per task into a
  per-task PSUM bank (PSUM start=True clears has_written for the whole
  bank, so interleaved chains in one bank are not allowed).
- The 25x25 solve runs as 4-task batches stacked at 32-partition stride:
  diagonal 32x32 tile_position matmuls in fp32r with 32-wide padding
  (zeros propagate, so no PSUM garbage reads), DVE does the batched
  elementwise updates.
"""

import numpy as np
import ml_dtypes
import concourse.bass as bass
import concourse.mybir as mybir
import concourse.tile as tile
from concourse import bacc
from concourse.bass_utils import run_bass_kernel_spmd

F32 = mybir.dt.float32
F32R = mybir.dt.float32r
BF16 = mybir.dt.bfloat16

N_CORES = 8
TASKS = 1024
TPC = TASKS // N_CORES  # tasks per core
D = 2048
NQ = 75
NS = 25
NW = 5
NCH = D // 128  # d-chunks
GRP = 4  # tasks per group
SQW = NS + NQ  # 100 cols per (task, chunk) in SQT
TW = NCH * SQW  # cols per task in SQT
NSP = 32  # padded solve width
NWP = 6  # padded n_way (fp32r matmul needs even free dim)
SOLVE_C = 1.0 / 3200.0
SOLVE_ITERS = 6

_cached = {}


def _host_consts():
    ident = np.eye(128, dtype=np.float32).astype(ml_dtypes.bfloat16)
    iblk1 = np.zeros((128, NSP), np.float32)
    iblk2 = np.zeros((128, NSP), np.float32)
    iblk2c = np.zeros((128, NSP), np.float32)
    for tau in range(GRP):
        for i in range(NS):
            iblk1[32 * tau + i, i] = 1.0
            iblk2[32 * tau + i, i] = 2.0
            iblk2c[32 * tau + i, i] = 2.0 * SOLVE_C
    return {"ident": ident, "iblk1": iblk1, "iblk2": iblk2, "iblk2c": iblk2c}


def _build_kernel(tasks: int):
    assert tasks % GRP == 0
    ngroups = tasks // GRP
    nc = bacc.Bacc("TRN2")

    query = nc.declare_dram_parameter("query", [tasks, NQ, D], F32, isOutput=False)
    support = nc.declare_dram_parameter("support", [tasks, NS, D], F32, isOutput=False)
    y_oh = nc.declare_dram_parameter("y_oh", [tasks, NS, NWP], BF16, isOutput=False)
    ident_d = nc.declare_dram_parameter("ident", [128, 128], BF16, isOutput=False)
    iblk1_d = nc.declare_dram_parameter("iblk1", [128, NSP], F32, isOutput=False)
    iblk2_d = nc.declare_dram_parameter("iblk2", [128, NSP], F32, isOutput=False)
    iblk2c_d = nc.declare_dram_parameter("iblk2c", [128, NSP], F32, isOutput=False)
    out_d = nc.declare_dram_parameter("out", [tasks, NQ, NW], F32, isOutput=True)

    cc = SOLVE_C

    with tile.TileContext(nc) as tc:
        with (
            tc.tile_pool(name="const", bufs=1) as const_p,
            tc.tile_pool(name="sraw", bufs=2) as sraw_p,
            tc.tile_pool(name="sb16", bufs=2) as sb16_p,
            tc.tile_pool(name="sqt", bufs=2) as sqt_p,
            tc.tile_pool(name="qraw", bufs=3) as qraw_p,
            tc.tile_pool(name="qb16", bufs=3) as qb16_p,
            tc.tile_pool(name="ysb", bufs=2) as y_p,
            tc.tile_pool(name="slvsb", bufs=2) as slvsb_p,
            tc.tile_pool(name="ct", bufs=8) as ct_p,
            tc.tile_pool(name="zsb", bufs=8) as z_p,
            tc.tile_pool(name="osb", bufs=8) as o_p,
            tc.tile_pool(name="tp", bufs=2, space="PSUM") as tp_ps,
            tc.tile_pool(name="gc", bufs=2, space="PSUM") as gc_ps,
            tc.tile_pool(name="slv", bufs=2, space="PSUM") as slv_ps,
            tc.tile_pool(name="fin", bufs=2, space="PSUM") as fin_ps,
        ):
            ident = const_p.tile([128, 128], BF16)
            nc.sync.dma_start(out=ident, in_=ident_d[:, :])
            iblk1 = const_p.tile([128, NSP], F32)
            nc.sync.dma_start(out=iblk1, in_=iblk1_d[:, :])
            iblk2 = const_p.tile([128, NSP], F32)
            nc.sync.dma_start(out=iblk2, in_=iblk2_d[:, :])
            iblk2c = const_p.tile([128, NSP], F32)
            nc.sync.dma_start(out=iblk2c, in_=iblk2c_d[:, :])

            for g in range(ngroups):
                t0 = g * GRP
                s_raw = sraw_p.tile([128, D], F32)
                nc.sync.dma_start(
                    out=s_raw[0 : GRP * NS, :],
                    in_=support[t0 : t0 + GRP].rearrange("a b c -> (a b) c"),
                )
                s16 = sb16_p.tile([128, D], BF16)
                nc.gpsimd.tensor_copy(s16[0 : GRP * NS, :], s_raw[0 : GRP * NS, :])
                sqt = sqt_p.tile([128, GRP * TW], BF16)
                sqt4 = sqt.rearrange("p (t c w) -> p t c w", t=GRP, c=NCH)
                for c in range(NCH):
                    pt = tp_ps.tile([128, SQW], BF16, tag="tp")
                    nc.tensor.transpose(
                        pt,
                        s16[0 : GRP * NS, c * 128 : (c + 1) * 128],
                        ident[0 : GRP * NS, 0 : GRP * NS],
                    )
                    nc.vector.tensor_copy(
                        sqt4[:, :, c, 0:NS],
                        pt.rearrange("p (t n) -> p t n", t=GRP),
                    )

                y_sb = y_p.tile([128, NWP], BF16)
                for tau in range(GRP):
                    nc.sync.dma_start(
                        out=y_sb[32 * tau : 32 * tau + NS, :], in_=y_oh[t0 + tau]
                    )

                gcs = []
                for tau in range(GRP):
                    t = t0 + tau
                    q_raw = qraw_p.tile([NQ, D], F32)
                    nc.sync.dma_start(out=q_raw, in_=query[t])
                    q16 = qb16_p.tile([NQ, D], BF16)
                    nc.gpsimd.tensor_copy(q16, q_raw)
                    for c in range(NCH):
                        pq = tp_ps.tile([128, SQW], BF16, tag="tp")
                        nc.tensor.transpose(
                            pq[:, 0:NQ],
                            q16[:, c * 128 : (c + 1) * 128],
                            ident[0:NQ, 0:NQ],
                        )
                        if c % 2 == 0:
                            nc.vector.tensor_copy(
                                sqt4[:, tau, c, NS:SQW], pq[:, 0:NQ]
                            )
                        else:
                            nc.scalar.copy(sqt4[:, tau, c, NS:SQW], pq[:, 0:NQ])

                    gc = gc_ps.tile([NS, SQW], F32)
                    for c in range(NCH):
                        nc.tensor.matmul(
                            gc,
                            lhsT=sqt4[:, tau, c, 0:NS],
                            rhs=sqt4[:, tau, c, :],
                            start=(c == 0),
                            stop=(c == NCH - 1),
                        )
                    gcs.append(gc)

                a_sb = slvsb_p.tile([128, NSP], BF16, tag="a")
                nc.vector.tensor_copy(a_sb, iblk1)
                for tau in range(GRP):
                    sl = slice(32 * tau, 32 * tau + NS)
                    nc.vector.tensor_add(
                        a_sb[sl, 0:NS], gcs[tau][:, 0:NS], iblk1[sl, 0:NS]
                    )
                x = slvsb_p.tile([128, NSP], BF16, tag="x")
                nc.vector.tensor_scalar_mul(x, a_sb, -cc * cc)
                nc.vector.tensor_add(x, x, iblk2c)
                cts = []
                for tau in range(GRP):
                    ct = ct_p.tile([NS, NQ], BF16)
                    nc.scalar.copy(ct, gcs[tau][:, NS:SQW])
                    cts.append(ct)

                for it in range(SOLVE_ITERS):
                    r = slv_ps.tile([128, NSP], F32, tag="slv")
                    for tau in range(GRP):
                        ksl = slice(32 * tau, 32 * tau + NS)
                        osl = slice(32 * tau, 32 * tau + 32)
                        nc.tensor.matmul(
                            r[osl, :],
                            lhsT=a_sb[ksl, :],
                            rhs=x[ksl, :],
                            start=True,
                            stop=True,
                            skip_group_check=True,
                            tile_position=(32 * tau, 32 * tau),
                        )
                    t_sb = slvsb_p.tile([128, NSP], BF16, tag="t")
                    nc.vector.tensor_sub(t_sb, iblk2, r)
                    x2 = slv_ps.tile([128, NSP], F32, tag="slv")
                    for tau in range(GRP):
                        ksl = slice(32 * tau, 32 * tau + NS)
                        osl = slice(32 * tau, 32 * tau + 32)
                        nc.tensor.matmul(
                            x2[osl, :],
                            lhsT=x[ksl, :],
                            rhs=t_sb[ksl, :],
                            start=True,
                            stop=True,
                            skip_group_check=True,
                            tile_position=(32 * tau, 32 * tau),
                        )
                    x = slvsb_p.tile([128, NSP], BF16, tag="x")
                    nc.vector.tensor_copy(x, x2)

                zp = slv_ps.tile([128, NWP], F32, tag="slv")
                for tau in range(GRP):
                    ksl = slice(32 * tau, 32 * tau + NS)
                    osl = slice(32 * tau, 32 * tau + 32)
                    nc.tensor.matmul(
                        zp[osl, :],
                        lhsT=x[ksl, :],
                        rhs=y_sb[ksl, :],
                        start=True,
                        stop=True,
                        skip_group_check=True,
                        tile_position=(32 * tau, 32 * tau),
                    )

                for tau in range(GRP):
                    t = t0 + tau
                    z_sb = z_p.tile([NS, NWP], BF16)
                    nc.scalar.copy(z_sb, zp[32 * tau : 32 * tau + NS, :])
                    op = fin_ps.tile([NQ, NWP], F32)
                    nc.tensor.matmul(
                        op,
                        lhsT=cts[tau],
                        rhs=z_sb,
                        start=True,
                        stop=True,
                    )
                    o_sb = o_p.tile([NQ, NW], F32)
                    nc.scalar.copy(o_sb, op[:, 0:NW])
                    nc.sync.dma_start(out=out_d[t], in_=o_sb)

    nc.compile()
    return nc


def kernel(query, support, support_labels, n_way, n_shot, **_ignored):
    query = np.ascontiguousarray(np.asarray(query, dtype=np.float32))
    support = np.ascontiguousarray(np.asarray(support, dtype=np.float32))
    labels = np.asarray(support_labels).astype(np.int64)
    n_way = int(n_way)
    n_shot = int(n_shot)
    assert n_way == NW and n_shot == 5
    assert query.shape == (TASKS, NQ, D) and support.shape == (TASKS, NS, D)

    # one-hot encode labels host-side (lossless relabeling of the int input),
    # padded to NWP=6 cols (fp32r matmul needs an even free dim)
    y = np.zeros((TASKS, NS, NWP), np.float32)
    np.put_along_axis(y, labels[:, :, None].astype(np.int64), 1.0, axis=2)
    y = y.astype(ml_dtypes.bfloat16)

    if "nc" not in _cached:
        _cached["nc"] = _build_kernel(TPC)
    nc = _cached["nc"]

    consts = _host_consts()
    in_maps = []
    for c in range(N_CORES):
        sl = slice(c * TPC, (c + 1) * TPC)
        in_maps.append(
            {
                "query": query[sl],
                "support": support[sl],
                "y_oh": np.ascontiguousarray(y[sl]),
                **consts,
            }
        )

    res = run_bass_kernel_spmd(nc, in_maps, list(range(N_CORES)))
    out = np.concatenate(
        [res.results[c]["out"] for c in range(N_CORES)], axis=0
    ).astype(np.float32)
    return out
